# revision 16
# baseline (speedup 1.0000x reference)
"""DeepSeek-MoE Trainium2 kernel (8 NeuronCores, expert-parallel).

Strategy
--------
* Routing (sigmoid + grouped top-k, DeepSeek noaux_tc) is replicated on every
  core in fp32 (top-k margins in this regime are ~2e-5, so bf16 routing would
  flip expert selections).
* Dispatch/combine are dense one-hot matmuls built on-device from the routing
  result (no indirect DMA): rank-within-expert comes from an exclusive cumsum
  over tokens realized as a matmul with triangular/ones masks, and the one-hot
  dispatch matrix D[t, c] = (rank[t, e_slot] == c) is built with per-partition
  tensor_scalar(is_equal) against an iota row.
* Expert parallelism: 4 experts per core (load-balanced bin-packing computed
  on the host at call time from the actual routing), per-slot capacities are
  compile-time (multiples of 128 covering the observed loads + margin).
* Expert weights are downcast to bf16 on the host (halves HBM traffic; the
  2e-2 rel-err budget tolerates bf16 GEMMs). All matmul accumulation is fp32
  in PSUM.
* Shared experts are sharded over their intermediate dim (352 channels/core).
* Schedule: prologue (x load/transpose/logits), routing on DVE overlapped
  with the shared-expert GEMMs on PE, dispatch, then phase A = all four
  routed slots' gate_up -> act -> actT (kept in SBUF), then phase B per
  h-half: down-proj for all slots, one combine pass (all slots + shared)
  accumulated in PSUM, staged to DRAM as bf16, and a bf16 ReduceScatter per
  half -- RS(half0) overlaps the half1 down+combine so only one RS is an
  exposed tail. Core r returns tokens [64r, 64r+64); the host concatenates.
"""

import numpy as np
import ml_dtypes

T, H, E, K, I = 512, 2048, 32, 8, 1408
NG, TKG = 8, 4
RSF = 2.5
NCORES = 8
P = 128
ISH = 2 * I // NCORES  # 352: shared-expert intermediate slice per core
HT = H // P            # 16 h-tiles
TT = T // P            # 4 token tiles
IT = I // P            # 11 i-tiles
GS = E // NG           # 4 experts per group
BIG = 1.0e9

bf16 = ml_dtypes.bfloat16


# ----------------------------------------------------------------------------
# Host-side routing mirror (only used to pick expert->core assignment and
# compile-time slot capacities; the device re-computes routing exactly).
# ----------------------------------------------------------------------------
def _host_loads(x, gate_w, bias):
    logits = (x.astype(np.float32) @ gate_w.astype(np.float32)).astype(np.float32)
    scores = (1.0 / (1.0 + np.exp(-logits))).astype(np.float32)
    sb = scores + bias[None, :].astype(np.float32)
    g = sb.reshape(T, NG, GS)
    pair = [g[..., i] + g[..., j] for i in range(GS) for j in range(i + 1, GS)]
    grp = np.max(np.stack(pair, -1), -1)
    gmask = np.zeros((T, NG), np.float32)
    gw = grp.copy()
    for _ in range(TKG):
        mx = gw.max(-1, keepdims=True)
        eq = (gw == mx).astype(np.float32)
        gmask += eq
        gw -= eq * BIG
    emask = np.repeat(gmask, GS, axis=1)
    m = sb + (emask * BIG - BIG)
    kmask = np.zeros((T, E), np.float32)
    for _ in range(K):
        mx = m.max(-1, keepdims=True)
        eq = (m == mx).astype(np.float32)
        kmask += eq
        m -= eq * BIG
    return kmask.sum(0)


def _plan_slots(loads, margin=2):
    caps = (np.ceil((loads + margin) / P).astype(int) * P).clip(P, None)
    order = np.argsort(-(caps * 1000 + loads))
    groups = [[] for _ in range(NCORES)]
    gsum = [0] * NCORES
    for e in order:
        cand = [i for i in sorted(range(NCORES), key=lambda i: (gsum[i], len(groups[i])))
                if len(groups[i]) < 4]
        i = cand[0]
        groups[i].append(int(e))
        gsum[i] += caps[e]
    for i in range(NCORES):
        groups[i].sort(key=lambda e: -caps[e])
    slot_caps = [int(max(caps[groups[i][j]] for i in range(NCORES))) for j in range(4)]
    return groups, slot_caps


# ----------------------------------------------------------------------------
# Device program
# ----------------------------------------------------------------------------
def _build_nc(slot_caps, single_core=False):
    import concourse.mybir as mybir
    import concourse.tile as tile
    from concourse import bacc
    from contextlib import ExitStack

    f32 = mybir.dt.float32
    b16 = mybir.dt.bfloat16
    Alu = mybir.AluOpType
    Act = mybir.ActivationFunctionType
    Ax = mybir.AxisListType

    cts = [c // P for c in slot_caps]            # ctiles per slot
    offs = np.cumsum([0] + slot_caps).tolist()   # D column offsets
    DCOLS = offs[-1]
    NCT = sum(cts)                               # total ctiles on this core
    cbase = np.cumsum([0] + cts).tolist()        # global ctile index base per slot
    CAPMAX = max(slot_caps)

    nc = bacc.Bacc("TRN2", target_bir_lowering=False, debug=False,
                   num_devices=1 if single_core else NCORES)

    # ---- I/O ----
    x_d = nc.dram_tensor("x", [T, H], f32, kind="ExternalInput")
    gw_d = nc.dram_tensor("gate_w", [H, E], f32, kind="ExternalInput")
    bias_d = nc.dram_tensor("bias_b", [P, E], f32, kind="ExternalInput")
    w13_d = nc.dram_tensor("w13s", [4, H, 2 * I], b16, kind="ExternalInput")
    w2_d = nc.dram_tensor("w2s", [4, I, H], b16, kind="ExternalInput")
    wgu_d = nc.dram_tensor("wgu_sh", [H, 2 * ISH], b16, kind="ExternalInput")
    wdn_d = nc.dram_tensor("wdn_sh", [ISH, H], b16, kind="ExternalInput")
    sel_d = nc.dram_tensor("sel", [E, 4], f32, kind="ExternalInput")
    iota_d = nc.dram_tensor("iota_r", [P, CAPMAX], f32, kind="ExternalInput")
    triu_d = nc.dram_tensor("triu_b", [P, P], b16, kind="ExternalInput")
    ones_d = nc.dram_tensor("ones_b", [P, P], b16, kind="ExternalInput")
    id32_d = nc.dram_tensor("id_f32", [P, P], f32, kind="ExternalInput")
    id16_d = nc.dram_tensor("id_b16", [P, P], b16, kind="ExternalInput")
    out_d = nc.dram_tensor("out_slice",
                           [T, H] if single_core else [T // NCORES, H], f32,
                           kind="ExternalOutput")

    partial_d = [nc.dram_tensor(f"partial{i}", [T, H // 2], b16,
                                kind="Internal") for i in range(2)]
    rs_d = [nc.dram_tensor(f"rs_out{i}", [T // NCORES, H // 2], b16,
                           kind="Internal") for i in range(2)]

    def cp(i, out, in_):
        # alternate psum/sbuf copies between DVE and ACT to balance engines
        if i % 2 == 0:
            nc.vector.tensor_copy(out=out, in_=in_)
        else:
            nc.scalar.copy(out, in_)

    xr = x_d.ap().rearrange("(tt p) h -> p tt h", p=P)
    gwr = gw_d.ap().rearrange("(ko p) e -> p ko e", p=P)
    w13r = w13_d.ap().rearrange("j (ko p) f -> j p ko f", p=P)
    w2r = w2_d.ap().rearrange("j (ko p) h -> j p ko h", p=P)
    wgur = wgu_d.ap().rearrange("(ko p) f -> p ko f", p=P)

    with tile.TileContext(nc) as tc, ExitStack() as ctx:
        pc = ctx.enter_context(tc.tile_pool(name="persist", bufs=1))
        xp = ctx.enter_context(tc.tile_pool(name="xstream", bufs=2))
        wp = ctx.enter_context(tc.tile_pool(name="wstream", bufs=3))
        ap_ = ctx.enter_context(tc.tile_pool(name="acts", bufs=1))
        tp_ = ctx.enter_context(tc.tile_pool(name="tmps", bufs=2))
        sp = ctx.enter_context(tc.tile_pool(name="smalls", bufs=2))
        psA = ctx.enter_context(tc.tile_pool(name="psumA", bufs=2, space="PSUM"))
        psB = ctx.enter_context(tc.tile_pool(name="psumB", bufs=1, space="PSUM"))
        op_ = ctx.enter_context(tc.tile_pool(name="ostage", bufs=2))

        def mmw(k, name):
            # two rotating 2-bank wide accumulators
            return psB.tile([P, 1024], f32, tag=f"mmw{k % 2}", name=name)

        # ---- small constant loads ----
        gw_sb = pc.tile([P, HT, E], f32, tag="gw")
        nc.sync.dma_start(gw_sb[:], gwr)
        bias_sb = pc.tile([P, E], f32, tag="bias")
        nc.sync.dma_start(bias_sb[:], bias_d.ap())
        sel_sb = pc.tile([E, 4], f32, tag="sel")
        nc.sync.dma_start(sel_sb[:], sel_d.ap())
        iota_sb = pc.tile([P, CAPMAX], f32, tag="iota")
        nc.sync.dma_start(iota_sb[:], iota_d.ap())
        triu_sb = pc.tile([P, P], b16, tag="triu")
        nc.sync.dma_start(triu_sb[:], triu_d.ap())
        ones_sb = pc.tile([P, P], b16, tag="ones")
        nc.sync.dma_start(ones_sb[:], ones_d.ap())
        id32_sb = pc.tile([P, P], f32, tag="id32")
        nc.sync.dma_start(id32_sb[:], id32_d.ap())
        id16_sb = pc.tile([P, P], b16, tag="id16")
        nc.sync.dma_start(id16_sb[:], id16_d.ap())

        # ---- stream x in 512-col chunks: cast to bf16, build x^T (PE), logits.
        # Transposes are batched 4-per-PSUM-tile with one wide copy out, and
        # the logits accumulate across all 16 h-tiles in a dedicated PSUM
        # tile that sigmoid reads directly (no DVE adds).
        x_bf = pc.tile([P, TT, H], b16, tag="xb")
        xT_bf = pc.tile([P, HT, T], b16, tag="xTb")
        lg_sb = pc.tile([P, TT, E], f32, tag="lg")
        for hc in range(8):
            xf = xp.tile([P, TT, 256], f32, tag="xf")
            nc.sync.dma_start(xf[:], xr[:, :, hc * 256:(hc + 1) * 256])
            cp(hc, x_bf[:, :, hc * 256:(hc + 1) * 256], xf[:])
            xtf = xp.tile([P, 2, T], f32, tag="xtf")  # [hp, ho_local, t]
            for hl in range(2):
                ptb = psA.tile([P, 512], f32, tag="sm", name="ptb")
                for tt in range(TT):
                    nc.tensor.transpose(ptb[:, tt * P:(tt + 1) * P],
                                        xf[:, tt, hl * P:(hl + 1) * P],
                                        id32_sb[:])
                cp(hl, xtf[:, hl, :], ptb[:])
                cp(hl + 1, xT_bf[:, hc * 2 + hl, :], ptb[:])
            pl = psA.tile([P, TT, E], f32, tag="pl", name="pl", bufs=1)
            for tt in range(TT):
                for hl in range(2):
                    nc.tensor.matmul(pl[:, tt, :],
                                     xtf[:, hl, tt * P:(tt + 1) * P],
                                     gw_sb[:, hc * 2 + hl, :],
                                     start=(hl == 0), stop=(hl == 1))
            if hc == 0:
                nc.vector.tensor_copy(out=lg_sb[:], in_=pl[:])
            else:
                nc.vector.tensor_tensor(lg_sb[:], lg_sb[:], pl[:], Alu.add)

        # ---- routing (fp32, on [P, TT, NG, GS] layouts), pure DVE/ACT work.
        # Emitted BEFORE the shared-expert GEMMs so the PE chews on those
        # matmuls while DVE computes the top-k masks.
        scores = pc.tile([P, TT, NG, GS], f32, tag="scores")
        nc.scalar.activation(scores.rearrange("p t g s -> p t (g s)"), lg_sb[:],
                             Act.Sigmoid)
        sbb = pc.tile([P, TT, NG, GS], f32, tag="sbb")
        nc.vector.tensor_tensor(
            sbb[:], scores[:],
            bias_sb.rearrange("p (g s) -> p g s", g=NG)[:, None, :, :]
            .to_broadcast([P, TT, NG, GS]), Alu.add)

        grp = sp.tile([P, TT, NG], f32, tag="grp")
        pw = sp.tile([P, TT, NG], f32, tag="pw")
        first = True
        for i in range(GS):
            for j in range(i + 1, GS):
                dst = grp if first else pw
                nc.vector.tensor_tensor(dst[:], sbb[:, :, :, i], sbb[:, :, :, j],
                                        Alu.add)
                if not first:
                    nc.vector.tensor_tensor(grp[:], grp[:], pw[:], Alu.max)
                first = False

        gmask = sp.tile([P, TT, NG], f32, tag="gmask")
        tmpg = sp.tile([P, TT, NG], f32, tag="tmpg")
        mxg = sp.tile([P, TT], f32, tag="mxg")
        for r in range(TKG):
            nc.vector.reduce_max(mxg[:], grp[:], axis=Ax.X)
            nc.vector.tensor_tensor(tmpg[:], grp[:],
                                    mxg[:, :, None].to_broadcast([P, TT, NG]),
                                    Alu.is_equal)
            if r == 0:
                nc.vector.tensor_copy(out=gmask[:], in_=tmpg[:])
            else:
                nc.vector.tensor_tensor(gmask[:], gmask[:], tmpg[:], Alu.add)
            if r < TKG - 1:
                nc.vector.tensor_scalar(tmpg[:], tmpg[:], BIG, None, Alu.mult)
                nc.vector.tensor_tensor(grp[:], grp[:], tmpg[:], Alu.subtract)

        m_t = pc.tile([P, TT, NG, GS], f32, tag="mt")
        nc.vector.tensor_scalar(m_t[:], gmask[:, :, :, None]
                                .to_broadcast([P, TT, NG, GS]),
                                BIG, -BIG, Alu.mult, Alu.add)
        nc.vector.tensor_tensor(m_t[:], m_t[:], sbb[:], Alu.add)
        m_f = m_t.rearrange("p t g s -> p t (g s)")

        kmask = pc.tile([P, TT, E], f32, tag="kmask")
        tmpk = sp.tile([P, TT, E], f32, tag="tmpk")
        mxk = sp.tile([P, TT], f32, tag="mxk")
        for r in range(K):
            nc.vector.reduce_max(mxk[:], m_f, axis=Ax.X)
            nc.vector.tensor_tensor(tmpk[:], m_f,
                                    mxk[:, :, None].to_broadcast([P, TT, E]),
                                    Alu.is_equal)
            if r == 0:
                nc.vector.tensor_copy(out=kmask[:], in_=tmpk[:])
            else:
                nc.vector.tensor_tensor(kmask[:], kmask[:], tmpk[:], Alu.add)
            if r < K - 1:
                nc.vector.tensor_scalar(tmpk[:], tmpk[:], BIG, None, Alu.mult)
                nc.vector.tensor_tensor(m_f, m_f, tmpk[:], Alu.subtract)

        wsel = sp.tile([P, TT, E], f32, tag="wsel")
        nc.vector.tensor_tensor(wsel[:], kmask[:],
                                scores.rearrange("p t g s -> p t (g s)"),
                                Alu.mult)
        denom = sp.tile([P, TT], f32, tag="denom")
        nc.vector.reduce_sum(denom[:], wsel[:], axis=Ax.X)
        winv = sp.tile([P, TT], f32, tag="winv")
        nc.vector.reciprocal(winv[:], denom[:])
        nc.vector.tensor_scalar(winv[:], winv[:], RSF, None, Alu.mult)
        W_t = pc.tile([P, TT, E], f32, tag="Wt")
        nc.vector.tensor_tensor(W_t[:], wsel[:],
                                winv[:, :, None].to_broadcast([P, TT, E]),
                                Alu.mult)

        count_bf = sp.tile([P, TT, E], b16, tag="countb")
        nc.scalar.copy(count_bf[:], kmask[:])
        namask = sp.tile([P, TT, E], f32, tag="namask")
        nc.vector.tensor_scalar(namask[:], kmask[:], -1.0e6, 1.0e6,
                                Alu.mult, Alu.add)

        # ---- shared expert gate/up (2 token-tile halves, wgu streamed).
        # Emitted here (after the routing DVE ops) so the PE queue has these
        # big matmuls to run while DVE grinds the top-k.
        act_sh = pc.tile([P, TT, ISH], b16, tag="actsh")
        for th in range(2):
            pshs = [mmw(2 * th + ttl, f"psh{ttl}") for ttl in range(2)]
            for kg in range(HT // 4):
                wguc = wp.tile([P, 4, 2 * ISH], b16, tag="wguc")
                nc.sync.dma_start(wguc[:], wgur[:, kg * 4:(kg + 1) * 4, :])
                for kl in range(4):
                    ko = kg * 4 + kl
                    for ttl in range(2):
                        tt = th * 2 + ttl
                        for q0 in range(0, 2 * ISH, 512):
                            qw = min(512, 2 * ISH - q0)
                            nc.tensor.matmul(
                                pshs[ttl][:, q0:q0 + qw],
                                xT_bf[:, ko, tt * P:(tt + 1) * P],
                                wguc[:, kl, q0:q0 + qw],
                                start=(ko == 0), stop=(ko == HT - 1))
            for ttl in range(2):
                tt = th * 2 + ttl
                tmpsh = tp_.tile([P, ISH], b16, tag="tmpsh")
                nc.scalar.activation(tmpsh[:], pshs[ttl][:, :ISH], Act.Silu)
                nc.vector.tensor_tensor(act_sh[:, tt, :], tmpsh[:],
                                        pshs[ttl][:, ISH:2 * ISH], Alu.mult)
        # transpose act_sh -> [i_s, t]  (3 transposes batched per PSUM tile;
        # partitions 96-127 of the third slice are never written nor read)
        actShT = pc.tile([P, 3, T], b16, tag="actShT")
        for tt in range(TT):
            pt3 = psA.tile([P, 384], b16, tag="sm", name="pt3")
            for io in range(3):
                iw = min(P, ISH - io * P)
                nc.tensor.transpose(pt3[:iw, io * P:(io + 1) * P],
                                    act_sh[:, tt, io * P:io * P + iw],
                                    id16_sb[:])
            cp(tt, actShT[:, :, tt * P:(tt + 1) * P],
               pt3.rearrange("p (a b) -> p a b", a=3))

        # ---- rank-within-expert cumsum (PE) and per-core expert selection ----
        baseA = pc.tile([P, TT, E], f32, tag="baseA")
        for mt in range(TT):
            pb = psA.tile([P, E], f32, tag="sm", name="pb")
            for kk in range(mt + 1):
                lhs = ones_sb if kk < mt else triu_sb
                nc.tensor.matmul(pb[:], lhs[:], count_bf[:, kk, :],
                                 start=(kk == 0), stop=(kk == mt))
            nc.vector.tensor_tensor(baseA[:, mt, :], pb[:], namask[:, mt, :],
                                    Alu.add)

        # transpose baseA, W -> [E, t]; select this core's 4 experts via sel
        baT = pc.tile([E, TT, P], f32, tag="baT")
        wT = pc.tile([E, TT, P], f32, tag="wT")
        pt1 = psA.tile([E, 512], f32, tag="sm", name="pt1")
        for tt in range(TT):
            nc.tensor.transpose(pt1[:, tt * P:(tt + 1) * P], baseA[:, tt, :],
                                id32_sb[:])
        nc.vector.tensor_copy(out=baT[:], in_=pt1.rearrange(
            "p (a b) -> p a b", a=TT))
        pt2 = psA.tile([E, 512], f32, tag="sm", name="pt2")
        for tt in range(TT):
            nc.tensor.transpose(pt2[:, tt * P:(tt + 1) * P], W_t[:, tt, :],
                                id32_sb[:])
        nc.scalar.copy(wT[:], pt2.rearrange("p (a b) -> p a b", a=TT))
        bsel = pc.tile([P, TT, 4], f32, tag="bsel")
        wsel4 = pc.tile([P, TT, 4], f32, tag="wsel4")
        for tt in range(TT):
            pb4 = psA.tile([P, 4], f32, tag="sm", name="pb4")
            nc.tensor.matmul(pb4[:], baT[:, tt, :], sel_sb[:], start=True,
                             stop=True)
            nc.vector.tensor_copy(out=bsel[:, tt, :], in_=pb4[:])
            pw4 = psA.tile([P, 4], f32, tag="sm", name="pw4")
            nc.tensor.matmul(pw4[:], wT[:, tt, :], sel_sb[:], start=True,
                             stop=True)
            nc.scalar.copy(wsel4[:, tt, :], pw4[:])

        # dispatch one-hot D (bf16); combine weights Wc built blockwise -> WcT
        D_sb = pc.tile([P, TT, DCOLS], b16, tag="D")
        WcT = pc.tile([P, NCT, T], b16, tag="WcT")
        for tt in range(TT):
            wcs = sp.tile([P, DCOLS], f32, tag="wcs")
            for j in range(4):
                cap = slot_caps[j]
                nc.vector.tensor_scalar(D_sb[:, tt, offs[j]:offs[j] + cap],
                                        iota_sb[:, :cap], bsel[:, tt, j:j + 1],
                                        None, Alu.is_equal)
                nc.vector.tensor_scalar(wcs[:, offs[j]:offs[j] + cap],
                                        iota_sb[:, :cap],
                                        bsel[:, tt, j:j + 1],
                                        wsel4[:, tt, j:j + 1],
                                        Alu.is_equal, Alu.mult)
            for cg in range(0, NCT, 4):
                cgw = min(4, NCT - cg)
                ptw = psA.tile([P, 512], f32, tag="sm", name="ptw")
                for cl in range(cgw):
                    nc.tensor.transpose(ptw[:, cl * P:(cl + 1) * P],
                                        wcs[:, (cg + cl) * P:(cg + cl + 1) * P],
                                        id32_sb[:])
                cp(cg + tt, WcT[:, cg:cg + cgw, tt * P:(tt + 1) * P],
                   ptw[:, :cgw * P].rearrange("p (a b) -> p a b", a=cgw))

        # ---- dispatch matmul: xeT[h, c] = sum_t x[t,h] D[t,c]  (one wide MM) ----
        xeT = pc.tile([P, HT, DCOLS], b16, tag="xeT")
        NDW = (DCOLS + 1023) // 1024
        pctr = 0
        for ko in range(HT):
            for dch in range(NDW):
                cw = min(1024, DCOLS - dch * 1024)
                px = mmw(pctr, "px")
                pctr += 1
                for tt in range(TT):
                    for q0 in range(0, cw, 512):
                        qw = min(512, cw - q0)
                        nc.tensor.matmul(
                            px[:, q0:q0 + qw],
                            x_bf[:, tt, ko * P:(ko + 1) * P],
                            D_sb[:, tt, dch * 1024 + q0:dch * 1024 + q0 + qw],
                            start=(tt == 0), stop=(tt == TT - 1))
                cp(ko + dch, xeT[:, ko, dch * 1024:dch * 1024 + cw],
                   px[:, :cw])

        # ---- phase A: routed experts gate_up -> act -> actT (all slots) ----
        FCH = []
        fo = 0
        while fo < I:
            FCH.append((fo, min(512, I - fo)))
            fo += 512
        KG = 4  # ko-tiles per w13 DMA chunk
        # w13s is host-packed: per 512-f-chunk, gate|up columns adjacent
        actT = pc.tile([P, IT, NCT * P], b16, tag="actTall")
        for j in range(4):
            ct = cts[j]
            act = ap_.tile([P, 2, I], b16, tag="act", name="act")
            for fci, (fo, fw) in enumerate(FCH):
                co = 1024 * fci
                pgus = [mmw(pctr + ci, f"pgu{ci}") for ci in range(ct)]
                pctr += ct
                for kg in range(HT // KG):
                    wtag = "xf" if (fci * (HT // KG) + kg) % 2 == 0 else "xtf"
                    wg = xp.tile([P, KG, 1024], b16, tag=wtag, name="wg")
                    nc.gpsimd.dma_start(
                        wg[:, :, :2 * fw],
                        w13r[j][:, kg * KG:(kg + 1) * KG, co:co + 2 * fw])
                    for kl in range(KG):
                        ko = kg * KG + kl
                        for ci in range(ct):
                            lhs = xeT[:, ko,
                                      offs[j] + ci * P: offs[j] + (ci + 1) * P]
                            for q0 in range(0, 2 * fw, 512):
                                qw = min(512, 2 * fw - q0)
                                nc.tensor.matmul(pgus[ci][:, q0:q0 + qw], lhs,
                                                 wg[:, kl, q0:q0 + qw],
                                                 start=(ko == 0),
                                                 stop=(ko == HT - 1))
                for ci in range(ct):
                    tmpa = tp_.tile([P, 512], b16, tag="tmpact")
                    nc.scalar.activation(tmpa[:, :fw], pgus[ci][:, :fw], Act.Silu)
                    nc.vector.tensor_tensor(act[:, ci, fo:fo + fw],
                                            tmpa[:, :fw], pgus[ci][:, fw:2 * fw],
                                            Alu.mult)
            # transpose act -> actT [i, c]  (4 transposes batched per PSUM tile)
            for ci in range(ct):
                for g0 in range(0, IT, 4):
                    gw_ = min(4, IT - g0)
                    pt4 = psA.tile([P, 512], b16, tag="sm", name="pt4")
                    for il in range(gw_):
                        io = g0 + il
                        nc.tensor.transpose(pt4[:, il * P:(il + 1) * P],
                                            act[:, ci, io * P:(io + 1) * P],
                                            id16_sb[:])
                    cp(g0 + ci, actT[:, g0:g0 + gw_,
                                     (cbase[j] + ci) * P:
                                     (cbase[j] + ci + 1) * P],
                       pt4[:, :gw_ * P].rearrange("p (a b) -> p a b", a=gw_))

        # ---- phase B: per h-half, down-proj all slots then combine + RS ----
        # w2 chunks ride the xf/xtf buffer tags (free by now): 4 rotating
        # buffers give deep prefetch across the h0->h1 boundary.
        KOG = [(0, 3), (3, 3), (6, 3), (9, 2)]
        w2ctr = 0
        for hh in range(2):
            ye = ap_.tile([P, NCT, 1024], b16, tag="yehalf", name="ye")
            for j in range(4):
                ct = cts[j]
                pys = [mmw(pctr + ci, f"py{ci}") for ci in range(ct)]
                pctr += ct
                for (ko0, kn) in KOG:
                    w2c = xp.tile([P, 3, 1024], b16,
                                  tag="xf" if w2ctr % 2 == 0 else "xtf",
                                  name="w2c")
                    w2ctr += 1
                    nc.sync.dma_start(
                        w2c[:, :kn, :],
                        w2r[j][:, ko0:ko0 + kn,
                               hh * 1024:(hh + 1) * 1024])
                    for kl in range(kn):
                        ko = ko0 + kl
                        for ci in range(ct):
                            for q0 in (0, 512):
                                nc.tensor.matmul(
                                    pys[ci][:, q0:q0 + 512],
                                    actT[:, ko, (cbase[j] + ci) * P:
                                         (cbase[j] + ci + 1) * P],
                                    w2c[:, kl, q0:q0 + 512],
                                    start=(ko == 0), stop=(ko == IT - 1))
                for ci in range(ct):
                    cp(ci + j, ye[:, cbase[j] + ci, :], pys[ci][:])

            # shared-expert down weights for this half
            wdnc = xp.tile([P, 3, 1024], b16, tag="wdnc", name="wdnc")
            for io in range(3):
                iw = min(P, ISH - io * P)
                nc.sync.dma_start(
                    wdnc[:iw, io, :],
                    wdn_d.ap()[io * P:io * P + iw,
                               hh * 1024:(hh + 1) * 1024])

            # combine: po[t, h] = sum_c Wc[t,c] ye[c,h] + shared
            for tt in range(TT):
                po = mmw(pctr, f"po{pctr % 3}")
                pctr += 1
                for q, cb in enumerate(range(NCT)):
                    for q0 in (0, 512):
                        nc.tensor.matmul(
                            po[:, q0:q0 + 512],
                            WcT[:, cb, tt * P:(tt + 1) * P],
                            ye[:, cb, q0:q0 + 512],
                            start=(q == 0), stop=False)
                for io in range(3):
                    iw = min(P, ISH - io * P)
                    for q0 in (0, 512):
                        nc.tensor.matmul(
                            po[:, q0:q0 + 512],
                            actShT[:iw, io, tt * P:(tt + 1) * P],
                            wdnc[:iw, io, q0:q0 + 512],
                            start=False, stop=(io == 2))
                for sh in range(2):
                    stg = op_.tile([P, 512], b16, tag="ostg")
                    cp(tt + sh, stg[:], po[:, sh * 512:(sh + 1) * 512])
                    nc.scalar.dma_start(
                        partial_d[hh].ap()[tt * P:(tt + 1) * P,
                                           sh * 512:(sh + 1) * 512], stg[:])

            # cross-core reduce-scatter for this half (gpsimd queue only
            # carries w13 loads + the two RS ops, so RS(h0) does not block
            # any h1 work except RS(h1) itself).
            if not single_core:
                nc.gpsimd.collective_compute(
                    "ReduceScatter", Alu.add,
                    replica_groups=[list(range(NCORES))],
                    ins=[partial_d[hh].ap().opt()],
                    outs=[rs_d[hh].ap().opt()],
                )

        # ---- final output: load RS result (bf16), cast to fp32, store ----
        # Everything here depends on the collectives, so it lives entirely
        # on the gpsimd queue (after the RS ops); putting any of it on the
        # sync/scalar/vector queues lets the scheduler hoist it ahead of
        # phase-B h1 work, stalling that work on RS(h0) completion.
        TOUT = T if single_core else T // NCORES
        for hh in range(2):
            src = partial_d[hh] if single_core else rs_d[hh]
            for t0 in range(0, TOUT, P):
                tw = min(P, TOUT - t0)
                for c0 in (0, 512):
                    ob = op_.tile([P, 512], b16, tag="obf", name="ob",
                                  bufs=2)
                    nc.gpsimd.dma_start(ob[:tw, :],
                                        src.ap()[t0:t0 + tw, c0:c0 + 512])
                    of = op_.tile([P, 512], f32, tag="off", name="of",
                                  bufs=2)
                    nc.gpsimd.tensor_copy(out=of[:tw, :], in_=ob[:tw, :])
                    nc.gpsimd.dma_start(
                        out_d.ap()[t0:t0 + tw,
                                   hh * 1024 + c0:hh * 1024 + c0 + 512],
                        of[:tw, :])

    nc.compile()
    return nc


_NC_CACHE = {}


def _pack_inputs(x, gate_w, bias, w13, w2, sgu, sdn, groups, slot_caps):
    """Per-core in_maps. w13 is packed so each 512-wide f-chunk has its gate
    and up columns adjacent: [g0|u0|g1|u1|g2|u2] with chunk widths 512/512/384."""
    CAPMAX = max(slot_caps)
    iota = np.tile(np.arange(CAPMAX, dtype=np.float32), (P, 1))
    triu = np.triu(np.ones((P, P), np.float32), 1).astype(bf16)
    ones = np.ones((P, P), bf16)
    id32 = np.eye(P, dtype=np.float32)
    id16 = np.eye(P, dtype=np.float32).astype(bf16)
    bias_b = np.tile(bias[None, :], (P, 1)).astype(np.float32)

    def pack_w13(w):   # w: [H, 2I] fp32 -> packed bf16
        cols = []
        fo = 0
        while fo < I:
            fw = min(512, I - fo)
            cols.append(w[:, fo:fo + fw])
            cols.append(w[:, I + fo:I + fo + fw])
            fo += fw
        return np.ascontiguousarray(np.concatenate(cols, axis=1)).astype(bf16)

    in_maps = []
    for core in range(NCORES):
        sel = np.zeros((E, 4), np.float32)
        for j, e in enumerate(groups[core]):
            sel[e, j] = 1.0
        gsl = slice(core * ISH, (core + 1) * ISH)
        wgu_sh = np.concatenate(
            [sgu[:, gsl], sgu[:, 2 * I + core * ISH: 2 * I + (core + 1) * ISH]],
            axis=1).astype(bf16)
        in_maps.append({
            "x": x, "gate_w": gate_w, "bias_b": bias_b,
            "w13s": np.stack([pack_w13(w13[e]) for e in groups[core]]),
            "w2s": np.ascontiguousarray(w2[groups[core]]).astype(bf16),
            "wgu_sh": np.ascontiguousarray(wgu_sh),
            "wdn_sh": np.ascontiguousarray(
                sdn[core * ISH:(core + 1) * ISH, :]).astype(bf16),
            "sel": sel, "iota_r": iota, "triu_b": triu, "ones_b": ones,
            "id_f32": id32, "id_b16": id16,
        })
    return in_maps


def kernel(hidden_states, residual, gate_w, bias, w13, w2, shared_gate_up,
           shared_down):
    from concourse.bass_utils import run_bass_kernel_spmd

    x = np.ascontiguousarray(np.asarray(hidden_states, np.float32))
    gate_w = np.ascontiguousarray(np.asarray(gate_w, np.float32))
    bias = np.asarray(bias, np.float32)
    w13 = np.asarray(w13, np.float32)
    w2 = np.asarray(w2, np.float32)
    sgu = np.asarray(shared_gate_up, np.float32)
    sdn = np.asarray(shared_down, np.float32)

    loads = _host_loads(x, gate_w, bias)
    groups, slot_caps = _plan_slots(loads)

    key = tuple(slot_caps)
    if key not in _NC_CACHE:
        _NC_CACHE[key] = _build_nc(slot_caps)
    nc = _NC_CACHE[key]

    in_maps = _pack_inputs(x, gate_w, bias, w13, w2, sgu, sdn, groups,
                           slot_caps)
    res = run_bass_kernel_spmd(nc, in_maps, core_ids=list(range(NCORES)))
    out = np.concatenate([res.results[c]["out_slice"] for c in range(NCORES)],
                         axis=0)
    return out.astype(np.float32)
